# revision 19
# baseline (speedup 1.0000x reference)
"""Entmax-1.5 explainer kernel for Trainium2 (8 NeuronCores, data parallel).

Computes, for attention [64, 12, 12, 1, 8192] f32:
    logits = mean over heads of attention[:, -1, :, 0, :]   -> [64, 8192]
    p      = entmax15(logits) along the last axis            -> [64, 8192]
and returns (p, logits), matching the reference.

Strategy:
  - Host slices the last layer / query position and shards the 64 batch
    rows across 8 cores (8 rows each).  Per-core layout: partition
    p = row*16 + chunk, 512 floats each; heads are pre-packed into three
    chunked tensors (5/5/2 heads wide) so DMAs are few and plain 2-D.
  - Head sum via a short binary tree of wide vector adds, overlapped with
    the chunk DMAs.
  - entmax15 threshold tau solved by a monotone-safe Newton iteration on
    f(tau) = sum relu(z - tau)^2 - 1 (z = logits/2; shift-invariance
    makes max-subtraction unnecessary).  Per-row sums come via a
    block-diagonal ones matmul; the "-1" is folded in by a PSUM pre-seed
    matmul and the state update is one reciprocal + one fused
    (in0*scale + bias) + in1 DVE op.
  - tau0 = (mean of per-partition maxes) - 0.3: empirically below tau*
    for this distribution (margin ~+0.13) so Newton is monotone; even if
    above, Newton self-corrects.  6 iters reach the f32 noise floor; we
    run 7.
"""

import sys

sys.path.insert(0, "/opt/trn_rl_repo")

import numpy as np

import concourse.bass as bass
import concourse.tile as tile
from concourse import bacc, mybir
from concourse.bass_utils import run_bass_kernel_spmd

# Problem constants (hardcoded per spec)
B = 64          # batch
H = 12          # heads
S = 8192        # key length
NCORES = 8
R = B // NCORES  # rows per core = 8
CPR = 16         # partitions per row
F = S // CPR     # 512 free elems per partition
P = 128          # partitions used

NEWTON_ITERS = 4
TAU0_C = 0.2
CHUNKS = (4, 4, 4)  # heads per DMA chunk

FP32 = mybir.dt.float32


def build_nc():
    nc = bacc.Bacc("TRN2", target_bir_lowering=False, debug=False)

    xs = [
        nc.dram_tensor(f"x{j}", [P, ch * F], FP32, kind="ExternalInput")
        for j, ch in enumerate(CHUNKS)
    ]
    w = nc.dram_tensor("w", [P, P], FP32, kind="ExternalInput")
    p_out = nc.dram_tensor("p", [P, F], FP32, kind="ExternalOutput")
    l_out = nc.dram_tensor("logits", [P, F], FP32, kind="ExternalOutput")

    add = mybir.AluOpType.add
    mult = mybir.AluOpType.mult
    amax = mybir.AluOpType.max
    subtract = mybir.AluOpType.subtract

    with tile.TileContext(nc) as tc:
        with (
            tc.tile_pool(name="xh", bufs=1) as xh_pool,
            tc.tile_pool(name="persist", bufs=1) as persist,
            tc.tile_pool(name="scratch", bufs=2) as scratch,
            tc.tile_pool(name="small", bufs=3) as small,
            tc.tile_pool(name="psum", bufs=2, space="PSUM") as psum_pool,
        ):
            wt = persist.tile([P, P], FP32)
            ones1 = persist.tile([1, P], FP32)
            const01 = persist.tile([1, 2], FP32)
            nc.vector.memset(ones1[:], 1.0)
            nc.vector.memset(const01[:, 0:1], 0.0)
            nc.vector.memset(const01[:, 1:2], 1.0)

            # ---- load heads in chunks across three DGE rings; reduce each
            # chunk to a single [P, F] partial as it arrives, combining
            # eagerly so the vector engine overlaps the stream
            rings = [nc.sync, nc.scalar, nc.gpsimd]
            partials = []
            for j, ch in enumerate(CHUNKS):
                t = xh_pool.tile([P, ch * F], FP32, tag=f"x{j}")
                rings[j % 3].dma_start(t[:], xs[j].ap())
                if ch == 4:
                    f1 = scratch.tile([P, 2 * F], FP32, tag=f"f1_{j}")
                    nc.vector.tensor_add(
                        f1[:], t[:, 0 : 2 * F], t[:, 2 * F : 4 * F]
                    )
                    pj = scratch.tile([P, F], FP32, tag=f"pair{j}")
                    nc.vector.tensor_add(pj[:], f1[:, 0:F], f1[:, F : 2 * F])
                elif ch == 2:
                    pj = scratch.tile([P, F], FP32, tag=f"pair{j}")
                    nc.vector.tensor_add(pj[:], t[:, 0:F], t[:, F : 2 * F])
                else:
                    raise ValueError(ch)
                partials.append(pj)
                if j == 1:
                    c01 = scratch.tile([P, F], FP32, tag="c01")
                    nc.vector.tensor_add(c01[:], partials[0][:], partials[1][:])
                    partials = [c01]
                elif j >= 3 and j % 2 == 1:
                    cx = scratch.tile([P, F], FP32, tag=f"cx{j}")
                    nc.vector.tensor_add(cx[:], partials[-2][:], partials[-1][:])
                    partials = partials[:-2] + [cx]
            while len(partials) > 2:
                cy = scratch.tile([P, F], FP32, tag=f"cy{len(partials)}")
                nc.vector.tensor_add(cy[:], partials[0][:], partials[1][:])
                partials = [cy] + partials[2:]
            acc = persist.tile([P, F], FP32)
            nc.vector.tensor_add(acc[:], partials[0][:], partials[-1][:])
            nc.scalar.dma_start(wt[:], w.ap())

            # ---- tau0 = (mean over row's 16 partitions of per-partition max)/24 - C
            # (reduce+matmul first so z/zneg overlap the PE init matmul)
            pmaxc = small.tile([P, 1], FP32, tag="pmax")
            nc.vector.tensor_reduce(
                pmaxc[:], acc[:], axis=mybir.AxisListType.X, op=amax
            )
            s0 = psum_pool.tile([P, 1], FP32, tag="s0")
            nc.tensor.matmul(s0[:], wt[:], pmaxc[:], start=True, stop=True)

            # logits = acc/12 (scalar engine) -> DMA out; z = acc/24 (vector)
            logits_t = persist.tile([P, F], FP32)
            nc.scalar.mul(logits_t[:], acc[:], 1.0 / H)
            nc.sync.dma_start(l_out.ap(), logits_t[:])

            z = persist.tile([P, F], FP32)
            nc.vector.tensor_scalar_mul(z[:], acc[:], 1.0 / (2.0 * H))
            zneg = persist.tile([P, F], FP32)
            nc.vector.tensor_scalar_mul(zneg[:], acc[:], -1.0 / (2.0 * H))

            nt = persist.tile([P, 1], FP32)
            # nt = -tau0 = TAU0_C - S0/(16*24)
            nc.vector.tensor_scalar(
                nt[:], s0[:], -1.0 / (CPR * 2.0 * H), TAU0_C, op0=mult, op1=add
            )
            nt2 = persist.tile([P, 1], FP32)
            nc.vector.tensor_scalar_mul(nt2[:], nt[:], 2.0)

            # ---- Newton iterations
            for it in range(NEWTON_ITERS + 1):
                last = it == NEWTON_ITERS
                if last:
                    # split the final relu^2 pass so the first half's DMA
                    # overlaps the second half's compute
                    half = F // 2
                    r = scratch.tile([P, F], FP32, tag="r")
                    r2 = scratch.tile([P, F], FP32, tag="r2")
                    for lo, hi, ring in ((0, half, nc.sync), (half, F, nc.scalar)):
                        nc.vector.tensor_scalar(
                            r[:, lo:hi], z[:, lo:hi], nt[:], 0.0, op0=add, op1=amax
                        )
                        nc.vector.scalar_tensor_tensor(
                            r2[:, lo:hi], z[:, lo:hi], nt[:], r[:, lo:hi],
                            op0=add, op1=mult,
                        )
                        ring.dma_start(p_out.ap()[:, lo:hi], r2[:, lo:hi])
                    break
                r = scratch.tile([P, F], FP32, tag="r")
                # r = max(z + nt, 0)
                nc.vector.tensor_scalar(r[:], z[:], nt[:], 0.0, op0=add, op1=amax)
                r2 = scratch.tile([P, F], FP32, tag="r2")
                s12 = small.tile([P, 2], FP32, tag="s12")
                # r2n = (-z - nt) * r == -relu(z + nt)^2 ; accum -> -sum r^2
                nc.vector.scalar_tensor_tensor(
                    r2[:], zneg[:], nt[:], r[:], op0=subtract, op1=mult,
                    accum_out=s12[:, 1:2],
                )
                # scalar engine: relu(acc/12 + 2nt) = 2r from acc directly
                # (no dependency on r) ; accum 2*sum(r) into s12[:,0]
                scr = scratch.tile([P, F], FP32, tag="scr")
                nc.scalar.activation(
                    scr[:], acc[:], mybir.ActivationFunctionType.Relu,
                    bias=nt2[:], scale=1.0 / H, accum_out=s12[:, 0:1],
                )
                # per-row sums replicated to each partition; col1 pre-seeded
                # with +1 so S12[:,1] = 1 - sum r^2, S12[:,0] = 2 sum r
                S12 = psum_pool.tile([P, 2], FP32, tag="S12")
                nc.tensor.matmul(S12[:], ones1[:], const01[:], start=True, stop=False)
                nc.tensor.matmul(S12[:], wt[:], s12[:], start=False, stop=True)
                # nt -= (sum r^2 - 1)/(2 sum r):
                # rc = 1/(2 sum r); nt = (S12[:,1]*rc + 0) + nt
                rc = small.tile([P, 1], FP32, tag="rc")
                nc.vector.reciprocal(rc[:], S12[:, 0:1])
                nc.vector.affine_then_add(
                    nt[:], S12[:, 1:2], nt[:], scale=rc[:], bias=0.0
                )
                nc.vector.tensor_scalar_mul(nt2[:], nt[:], 2.0)

    nc.compile()
    return nc


_NC = None


def _get_nc():
    global _NC
    if _NC is None:
        _NC = build_nc()
    return _NC


def _make_w():
    return np.kron(np.eye(R, dtype=np.float32), np.ones((CPR, CPR), np.float32))


def shard_x(core_slice):
    # [R, H, S] -> dict of chunk tensors [P, ch*F], partition p = r*CPR + c,
    # chunk j holds heads offs[j]..offs[j]+ch-1 side by side in the free dim
    xh = np.ascontiguousarray(
        core_slice.reshape(R, H, CPR, F).transpose(1, 0, 2, 3).reshape(H, P, F)
    ).astype(np.float32, copy=False)
    out = {}
    off = 0
    for j, ch in enumerate(CHUNKS):
        blk = xh[off : off + ch]  # [ch, P, F]
        out[f"x{j}"] = np.ascontiguousarray(
            blk.transpose(1, 0, 2).reshape(P, ch * F)
        )
        off += ch
    return out


def unshard_out(arr):
    # [P, F] -> [R, S]
    return np.asarray(arr).reshape(R, CPR, F).reshape(R, S)


def _shards(attention):
    att = np.asarray(attention)
    sl = att[:, -1, :, 0, :]  # [64, 12, 8192]
    wmat = _make_w()
    maps = []
    for i in range(NCORES):
        m = shard_x(sl[i * R : (i + 1) * R])
        m["w"] = wmat
        maps.append(m)
    return maps


def _ensure_ntff_hook():
    """This image's antenv lacks axon_hooks; synthesize it from the boot
    agent's ctypes NTFF driver so trace=True can capture HW profiles."""
    import types

    try:
        from antenv import axon_hooks  # noqa: F401

        return
    except ImportError:
        pass
    import antenv  # noqa: F401
    from trn_agent_boot.trn_boot import _ntff_profile_via_ctypes

    mod = types.ModuleType("antenv.axon_hooks")
    hook = _ntff_profile_via_ctypes("/opt/axon/libaxon_pjrt.so")
    mod.get_axon_ntff_profile_hook = lambda: hook
    mod.set_axon_ntff_profile_hook = lambda h: None
    sys.modules["antenv.axon_hooks"] = mod

    # avoid the S3 artifact upload in the trace post-processing path
    import concourse.bass_utils as bu

    bu.upload_artifacts = lambda tmpdir: tmpdir


def run(attention, trace=False, **trace_kwargs):
    if trace:
        _ensure_ntff_hook()
    nc = _get_nc()
    res = run_bass_kernel_spmd(
        nc,
        _shards(attention),
        core_ids=list(range(NCORES)),
        trace=trace,
        **trace_kwargs,
    )
    p_full = np.concatenate(
        [unshard_out(res.results[i]["p"]) for i in range(NCORES)], axis=0
    )
    l_full = np.concatenate(
        [unshard_out(res.results[i]["logits"]) for i in range(NCORES)], axis=0
    )
    return (p_full, l_full), res


def kernel(attention):
    (p_full, l_full), _ = run(attention, trace=False)
    return p_full, l_full


# revision 20
# speedup vs baseline: 1.0364x; 1.0364x over previous
"""Entmax-1.5 explainer kernel for Trainium2 (8 NeuronCores, data parallel).

Computes, for attention [64, 12, 12, 1, 8192] f32:
    logits = mean over heads of attention[:, -1, :, 0, :]   -> [64, 8192]
    p      = entmax15(logits) along the last axis            -> [64, 8192]
and returns (p, logits), matching the reference.

Strategy:
  - Host slices the last layer / query position and shards the 64 batch
    rows across 8 cores (8 rows each).  Per-core layout: partition
    p = row*16 + chunk, 512 floats each; heads are pre-packed into three
    chunked tensors (5/5/2 heads wide) so DMAs are few and plain 2-D.
  - Head sum via a short binary tree of wide vector adds, overlapped with
    the chunk DMAs.
  - entmax15 threshold tau solved by a monotone-safe Newton iteration on
    f(tau) = sum relu(z - tau)^2 - 1 (z = logits/2; shift-invariance
    makes max-subtraction unnecessary).  Per-row sums come via a
    block-diagonal ones matmul; the "-1" is folded in by a PSUM pre-seed
    matmul and the state update is one reciprocal + one fused
    (in0*scale + bias) + in1 DVE op.
  - tau0 = (mean of per-partition maxes) - 0.3: empirically below tau*
    for this distribution (margin ~+0.13) so Newton is monotone; even if
    above, Newton self-corrects.  6 iters reach the f32 noise floor; we
    run 7.
"""

import sys

sys.path.insert(0, "/opt/trn_rl_repo")

import numpy as np

import concourse.bass as bass
import concourse.tile as tile
from concourse import bacc, mybir
from concourse.bass_utils import run_bass_kernel_spmd

# Problem constants (hardcoded per spec)
B = 64          # batch
H = 12          # heads
S = 8192        # key length
NCORES = 8
R = B // NCORES  # rows per core = 8
CPR = 16         # partitions per row
F = S // CPR     # 512 free elems per partition
P = 128          # partitions used

NEWTON_ITERS = 4
TAU0_C = 0.2
CHUNKS = (2, 2, 2, 2, 2, 2)  # heads per DMA chunk

FP32 = mybir.dt.float32


def build_nc():
    nc = bacc.Bacc("TRN2", target_bir_lowering=False, debug=False)

    xs = [
        nc.dram_tensor(f"x{j}", [P, ch * F], FP32, kind="ExternalInput")
        for j, ch in enumerate(CHUNKS)
    ]
    w = nc.dram_tensor("w", [P, P], FP32, kind="ExternalInput")
    p_out = nc.dram_tensor("p", [P, F], FP32, kind="ExternalOutput")
    l_out = nc.dram_tensor("logits", [P, F], FP32, kind="ExternalOutput")

    add = mybir.AluOpType.add
    mult = mybir.AluOpType.mult
    amax = mybir.AluOpType.max
    subtract = mybir.AluOpType.subtract

    with tile.TileContext(nc) as tc:
        with (
            tc.tile_pool(name="xh", bufs=1) as xh_pool,
            tc.tile_pool(name="persist", bufs=1) as persist,
            tc.tile_pool(name="scratch", bufs=2) as scratch,
            tc.tile_pool(name="small", bufs=3) as small,
            tc.tile_pool(name="psum", bufs=2, space="PSUM") as psum_pool,
        ):
            wt = persist.tile([P, P], FP32)
            ones1 = persist.tile([1, P], FP32)
            const01 = persist.tile([1, 2], FP32)
            nc.vector.memset(ones1[:], 1.0)
            nc.vector.memset(const01[:, 0:1], 0.0)
            nc.vector.memset(const01[:, 1:2], 1.0)

            # ---- load heads in chunks across three DGE rings; reduce each
            # chunk to a single [P, F] partial as it arrives, combining
            # eagerly so the vector engine overlaps the stream
            rings = [nc.sync, nc.scalar, nc.gpsimd]
            partials = []
            for j, ch in enumerate(CHUNKS):
                t = xh_pool.tile([P, ch * F], FP32, tag=f"x{j}")
                rings[j % 3].dma_start(t[:], xs[j].ap())
                if ch == 4:
                    f1 = scratch.tile([P, 2 * F], FP32, tag=f"f1_{j}")
                    nc.vector.tensor_add(
                        f1[:], t[:, 0 : 2 * F], t[:, 2 * F : 4 * F]
                    )
                    pj = scratch.tile([P, F], FP32, tag=f"pair{j}")
                    nc.vector.tensor_add(pj[:], f1[:, 0:F], f1[:, F : 2 * F])
                elif ch == 2:
                    pj = scratch.tile([P, F], FP32, tag=f"pair{j}")
                    nc.vector.tensor_add(pj[:], t[:, 0:F], t[:, F : 2 * F])
                else:
                    raise ValueError(ch)
                partials.append(pj)
                if j == 1:
                    c01 = scratch.tile([P, F], FP32, tag="c01")
                    nc.vector.tensor_add(c01[:], partials[0][:], partials[1][:])
                    partials = [c01]
                elif j >= 3 and j % 2 == 1:
                    cx = scratch.tile([P, F], FP32, tag=f"cx{j}")
                    nc.vector.tensor_add(cx[:], partials[-2][:], partials[-1][:])
                    partials = partials[:-2] + [cx]
            while len(partials) > 2:
                cy = scratch.tile([P, F], FP32, tag=f"cy{len(partials)}")
                nc.vector.tensor_add(cy[:], partials[0][:], partials[1][:])
                partials = [cy] + partials[2:]
            acc = persist.tile([P, F], FP32)
            nc.vector.tensor_add(acc[:], partials[0][:], partials[-1][:])
            nc.scalar.dma_start(wt[:], w.ap())

            # ---- tau0 = (mean over row's 16 partitions of per-partition max)/24 - C
            # (reduce+matmul first so z/zneg overlap the PE init matmul)
            pmaxc = small.tile([P, 1], FP32, tag="pmax")
            nc.vector.tensor_reduce(
                pmaxc[:], acc[:], axis=mybir.AxisListType.X, op=amax
            )
            s0 = psum_pool.tile([P, 1], FP32, tag="s0")
            nc.tensor.matmul(s0[:], wt[:], pmaxc[:], start=True, stop=True)

            # logits = acc/12 (scalar engine) -> DMA out; z = acc/24 (vector)
            logits_t = persist.tile([P, F], FP32)
            nc.scalar.mul(logits_t[:], acc[:], 1.0 / H)
            nc.sync.dma_start(l_out.ap(), logits_t[:])

            z = persist.tile([P, F], FP32)
            nc.vector.tensor_scalar_mul(z[:], acc[:], 1.0 / (2.0 * H))
            zneg = persist.tile([P, F], FP32)
            nc.vector.tensor_scalar_mul(zneg[:], acc[:], -1.0 / (2.0 * H))

            nt = persist.tile([P, 1], FP32)
            # nt = -tau0 = TAU0_C - S0/(16*24)
            nc.vector.tensor_scalar(
                nt[:], s0[:], -1.0 / (CPR * 2.0 * H), TAU0_C, op0=mult, op1=add
            )
            nt2 = persist.tile([P, 1], FP32)
            nc.vector.tensor_scalar_mul(nt2[:], nt[:], 2.0)

            # ---- Newton iterations
            for it in range(NEWTON_ITERS + 1):
                last = it == NEWTON_ITERS
                if last:
                    # split the final relu^2 pass so the first half's DMA
                    # overlaps the second half's compute
                    half = F // 2
                    r = scratch.tile([P, F], FP32, tag="r")
                    r2 = scratch.tile([P, F], FP32, tag="r2")
                    for lo, hi, ring in ((0, half, nc.sync), (half, F, nc.scalar)):
                        nc.vector.tensor_scalar(
                            r[:, lo:hi], z[:, lo:hi], nt[:], 0.0, op0=add, op1=amax
                        )
                        nc.vector.scalar_tensor_tensor(
                            r2[:, lo:hi], z[:, lo:hi], nt[:], r[:, lo:hi],
                            op0=add, op1=mult,
                        )
                        ring.dma_start(p_out.ap()[:, lo:hi], r2[:, lo:hi])
                    break
                r = scratch.tile([P, F], FP32, tag="r")
                # r = max(z + nt, 0)
                nc.vector.tensor_scalar(r[:], z[:], nt[:], 0.0, op0=add, op1=amax)
                r2 = scratch.tile([P, F], FP32, tag="r2")
                s12 = small.tile([P, 2], FP32, tag="s12")
                # r2n = (-z - nt) * r == -relu(z + nt)^2 ; accum -> -sum r^2
                nc.vector.scalar_tensor_tensor(
                    r2[:], zneg[:], nt[:], r[:], op0=subtract, op1=mult,
                    accum_out=s12[:, 1:2],
                )
                # scalar engine: relu(acc/12 + 2nt) = 2r from acc directly
                # (no dependency on r) ; accum 2*sum(r) into s12[:,0]
                scr = scratch.tile([P, F], FP32, tag="scr")
                nc.scalar.activation(
                    scr[:], acc[:], mybir.ActivationFunctionType.Relu,
                    bias=nt2[:], scale=1.0 / H, accum_out=s12[:, 0:1],
                )
                # per-row sums replicated to each partition; col1 pre-seeded
                # with +1 so S12[:,1] = 1 - sum r^2, S12[:,0] = 2 sum r
                S12 = psum_pool.tile([P, 2], FP32, tag="S12")
                nc.tensor.matmul(S12[:], ones1[:], const01[:], start=True, stop=False)
                nc.tensor.matmul(S12[:], wt[:], s12[:], start=False, stop=True)
                # nt -= (sum r^2 - 1)/(2 sum r):
                # rc = 1/(2 sum r); nt = (S12[:,1]*rc + 0) + nt
                rc = small.tile([P, 1], FP32, tag="rc")
                nc.vector.reciprocal(rc[:], S12[:, 0:1])
                nc.vector.affine_then_add(
                    nt[:], S12[:, 1:2], nt[:], scale=rc[:], bias=0.0
                )
                nc.vector.tensor_scalar_mul(nt2[:], nt[:], 2.0)

    nc.compile()
    return nc


_NC = None


def _get_nc():
    global _NC
    if _NC is None:
        _NC = build_nc()
    return _NC


def _make_w():
    return np.kron(np.eye(R, dtype=np.float32), np.ones((CPR, CPR), np.float32))


def shard_x(core_slice):
    # [R, H, S] -> dict of chunk tensors [P, ch*F], partition p = r*CPR + c,
    # chunk j holds heads offs[j]..offs[j]+ch-1 side by side in the free dim
    xh = np.ascontiguousarray(
        core_slice.reshape(R, H, CPR, F).transpose(1, 0, 2, 3).reshape(H, P, F)
    ).astype(np.float32, copy=False)
    out = {}
    off = 0
    for j, ch in enumerate(CHUNKS):
        blk = xh[off : off + ch]  # [ch, P, F]
        out[f"x{j}"] = np.ascontiguousarray(
            blk.transpose(1, 0, 2).reshape(P, ch * F)
        )
        off += ch
    return out


def unshard_out(arr):
    # [P, F] -> [R, S]
    return np.asarray(arr).reshape(R, CPR, F).reshape(R, S)


def _shards(attention):
    att = np.asarray(attention)
    sl = att[:, -1, :, 0, :]  # [64, 12, 8192]
    wmat = _make_w()
    maps = []
    for i in range(NCORES):
        m = shard_x(sl[i * R : (i + 1) * R])
        m["w"] = wmat
        maps.append(m)
    return maps


def _ensure_ntff_hook():
    """This image's antenv lacks axon_hooks; synthesize it from the boot
    agent's ctypes NTFF driver so trace=True can capture HW profiles."""
    import types

    try:
        from antenv import axon_hooks  # noqa: F401

        return
    except ImportError:
        pass
    import antenv  # noqa: F401
    from trn_agent_boot.trn_boot import _ntff_profile_via_ctypes

    mod = types.ModuleType("antenv.axon_hooks")
    hook = _ntff_profile_via_ctypes("/opt/axon/libaxon_pjrt.so")
    mod.get_axon_ntff_profile_hook = lambda: hook
    mod.set_axon_ntff_profile_hook = lambda h: None
    sys.modules["antenv.axon_hooks"] = mod

    # avoid the S3 artifact upload in the trace post-processing path
    import concourse.bass_utils as bu

    bu.upload_artifacts = lambda tmpdir: tmpdir


def run(attention, trace=False, **trace_kwargs):
    if trace:
        _ensure_ntff_hook()
    nc = _get_nc()
    res = run_bass_kernel_spmd(
        nc,
        _shards(attention),
        core_ids=list(range(NCORES)),
        trace=trace,
        **trace_kwargs,
    )
    p_full = np.concatenate(
        [unshard_out(res.results[i]["p"]) for i in range(NCORES)], axis=0
    )
    l_full = np.concatenate(
        [unshard_out(res.results[i]["logits"]) for i in range(NCORES)], axis=0
    )
    return (p_full, l_full), res


def kernel(attention):
    (p_full, l_full), _ = run(attention, trace=False)
    return p_full, l_full
